# revision 9
# baseline (speedup 1.0000x reference)
"""MoE (noisy top-2 routing, dense expert stack) on 8 Trainium2 NeuronCores.

Strategy: expert-parallel with host-side routing as the sharding step. The
top-2 routing decision is the sharding function: the host computes the noisy
gating in fp64 (bit-robust reproduction of the reference's fp32 top-2
selection), and ships each core exactly the tokens routed to its expert,
padded to a uniform tile count so all 8 cores run the same SPMD program.

Each core then, entirely on device:
  - recomputes the gating logits for its slots (fp32r matmuls) and turns them
    into the top-2 softmax weight of its own expert via a host-provided
    +1/-1 selector plane: w = sigmoid(sum_e noisy[e] * sel[e]) — the
    softmax-over-2 collapses to a sigmoid of the logit difference,
  - runs the expert FFN (relu(x@W1+b1) @ W2 + b2) as fp32r matmuls at full
    PE rate; layer 1 emits h transposed so the ReLU+bias is one activation
    and hT chains straight into layer 2 as lhsT,
  - scales each output row by w and stores the weighted rows.

The host scatter-adds the (at most 2) pre-weighted rows per token — the
"all-reduce of the weighted combine" of the expert-parallel sharding, done
as part of unsharding. Per-core compute drops to the routed ~2/8 of the
dense reference instead of all 8 experts on all tokens.
"""

import sys

sys.path.insert(0, "/opt/trn_rl_repo")

import numpy as np

import concourse.bass as bass
import concourse.mybir as mybir
import concourse.tile as tile
from concourse import bacc
from concourse.bass_utils import run_bass_kernel_spmd

N_CORES = 8
N, D, H, E = 8192, 1024, 2048, 8
P = 128
KD = D // P                 # 8  k-chunks over D
MH = H // P                 # 16 h-chunks

F32 = mybir.dt.float32
F32R = mybir.dt.float32r
AX = mybir.AxisListType
ALU = mybir.AluOpType
ACT_F = mybir.ActivationFunctionType


def _build(slots, repeat=1):
    """SPMD program for one core = one expert over `slots` routed tokens."""
    assert slots % 512 == 0
    NT = slots // 512

    nc = bacc.Bacc(None, target_bir_lowering=False, debug=False)

    xTg = nc.dram_tensor("xTg", [D, slots], F32, kind="ExternalInput")
    noiseg = nc.dram_tensor("noiseg", [slots, E], F32, kind="ExternalInput")
    ohd = nc.dram_tensor("ohd", [slots, E], F32, kind="ExternalInput")
    W1c = nc.dram_tensor("W1c", [D, H], F32, kind="ExternalInput")
    b1c = nc.dram_tensor("b1c", [H], F32, kind="ExternalInput")
    W2c = nc.dram_tensor("W2c", [H, D], F32, kind="ExternalInput")
    b2c = nc.dram_tensor("b2c", [D], F32, kind="ExternalInput")
    Wgc = nc.dram_tensor("Wgc", [D, E], F32, kind="ExternalInput")
    bgc = nc.dram_tensor("bgc", [E], F32, kind="ExternalInput")
    yc = nc.dram_tensor("yc", [slots, D], F32, kind="ExternalOutput")

    with tile.TileContext(nc) as tc:
        with (
            tc.tile_pool(name="persist", bufs=1) as persist,
            tc.tile_pool(name="xs", bufs=2) as xs,
            tc.tile_pool(name="w2s", bufs=3) as w2s,
            tc.tile_pool(name="gat", bufs=2) as gat,
            tc.tile_pool(name="yws", bufs=3) as yws,
            tc.tile_pool(name="pg", bufs=2, space="PSUM") as pg,
            tc.tile_pool(name="ph", bufs=2, space="PSUM") as ph,
            tc.tile_pool(name="py", bufs=4, space="PSUM") as py,
        ):
            # ---- persistent tiles ----
            W1_sb = persist.tile([P, KD, H], F32R)
            nc.sync.dma_start(
                W1_sb[:], W1c.rearrange("(kd p) h -> p kd h", p=P).bitcast(F32R)
            )
            Wg_sb = persist.tile([P, KD, E], F32R)
            nc.sync.dma_start(
                Wg_sb[:], Wgc.rearrange("(kd p) e -> p kd e", p=P).bitcast(F32R)
            )
            b1_sb = persist.tile([P, MH], F32)
            nc.sync.dma_start(b1_sb[:], b1c.rearrange("(m p) -> p m", p=P))
            b2r = persist.tile([P, D], F32)
            nc.sync.dma_start(b2r[:], b2c[None, :].to_broadcast((P, D)))
            bgr = persist.tile([P, E], F32)
            nc.sync.dma_start(bgr[:], bgc[None, :].to_broadcast((P, E)))

            hT = persist.tile([P, MH, 512], F32R)
            wcol = persist.tile([P, 4 * NT], F32)

            for _rep in range(repeat):
                for st in range(NT):
                    ss = slice(st * 512, (st + 1) * 512)
                    xg = xs.tile([P, KD, 512], F32R, tag="xg")
                    nc.sync.dma_start(
                        xg[:],
                        xTg[:, ss].rearrange("(kd p) t -> p kd t", p=P).bitcast(F32R),
                    )
                    noi = xs.tile([P, 4, E], F32, tag="noi")
                    nc.sync.dma_start(
                        noi[:], noiseg[ss, :].rearrange("(c p) e -> p c e", p=P)
                    )
                    oht = xs.tile([P, 4, E], F32, tag="oht")
                    nc.sync.dma_start(
                        oht[:], ohd[ss, :].rearrange("(c p) e -> p c e", p=P)
                    )

                    # gating: w = sigmoid(sum_e sel[e] * noisy[e])
                    for c4 in range(4):
                        ch = st * 4 + c4
                        cs = slice(c4 * P, (c4 + 1) * P)
                        g_ps = pg.tile([P, E], F32, tag="gps")
                        for kd in range(KD):
                            nc.tensor.matmul(
                                g_ps[:],
                                xg[:, kd, cs],
                                Wg_sb[:, kd, :],
                                start=(kd == 0),
                                stop=(kd == KD - 1),
                            )
                        noisy = gat.tile([P, E], F32, tag="noisy")
                        nz = gat.tile([P, E], F32, tag="nz")
                        nc.vector.scalar_tensor_tensor(
                            nz[:], noi[:, c4, :], 0.1, bgr[:], ALU.mult, ALU.add
                        )
                        nc.vector.tensor_tensor(noisy[:], g_ps[:], nz[:], ALU.add)
                        sel = gat.tile([P, E], F32, tag="sel")
                        nc.vector.tensor_tensor(
                            sel[:], noisy[:], oht[:, c4, :], ALU.mult
                        )
                        dd = gat.tile([P, 1], F32, tag="dd")
                        nc.vector.tensor_reduce(dd[:], sel[:], axis=AX.X, op=ALU.add)
                        nc.scalar.activation(wcol[:, ch : ch + 1], dd[:], ACT_F.Sigmoid)

                    # layer 1: hT = relu(W1^T-chunk @ x + b1), h on partitions
                    for m in range(MH):
                        h_ps = ph.tile([P, 512], F32, tag="hps")
                        for kd in range(KD):
                            nc.tensor.matmul(
                                h_ps[:],
                                W1_sb[:, kd, m * P : (m + 1) * P],
                                xg[:, kd, :],
                                start=(kd == 0),
                                stop=(kd == KD - 1),
                            )
                        nc.scalar.activation(
                            hT[:, m, :],
                            h_ps[:],
                            ACT_F.Relu,
                            bias=b1_sb[:, m : m + 1],
                        )

                    # layer 2 + bias + weight + store
                    for nh in range(2):
                        ns = slice(nh * 512, (nh + 1) * 512)
                        y_ps = [
                            py.tile([P, 512], F32, tag="yps", name=f"yps{i}")
                            for i in range(4)
                        ]
                        for khg in range(4):
                            w2t = w2s.tile([P, 4, 512], F32R, tag="w2t")
                            nc.sync.dma_start(
                                w2t[:],
                                W2c[khg * 512 : (khg + 1) * 512, ns]
                                .rearrange("(kh p) n -> p kh n", p=P)
                                .bitcast(F32R),
                            )
                            for c4 in range(4):
                                cs = slice(c4 * P, (c4 + 1) * P)
                                for kh in range(4):
                                    nc.tensor.matmul(
                                        y_ps[c4][:],
                                        hT[:, khg * 4 + kh, cs],
                                        w2t[:, kh, :],
                                        start=(khg == 0 and kh == 0),
                                        stop=(khg == 3 and kh == 3),
                                    )
                        for c4 in range(4):
                            ch = st * 4 + c4
                            yw = yws.tile([P, 512], F32, tag="yw")
                            nc.vector.tensor_tensor(
                                yw[:], y_ps[c4][:], b2r[:, ns], ALU.add
                            )
                            nc.vector.tensor_scalar(
                                yw[:], yw[:], wcol[:, ch : ch + 1], None, ALU.mult
                            )
                            nc.sync.dma_start(
                                yc[st * 512 + c4 * P : st * 512 + (c4 + 1) * P, ns],
                                yw[:],
                            )

    nc.compile()
    return nc


_NC_CACHE = {}


def _get_nc(slots, repeat=1):
    key = (slots, repeat)
    if key not in _NC_CACHE:
        _NC_CACHE[key] = _build(slots, repeat)
    return _NC_CACHE[key]


def prepare(x, W1, b1, W2, b2, Wg, bg, noise):
    """Host-side routing/sharding: fp64 noisy top-2, per-expert token lists,
    per-core input maps, and the scatter-add spec for unsharding."""
    x = np.ascontiguousarray(np.asarray(x, dtype=np.float32))
    noise = np.asarray(noise, dtype=np.float32)
    W1 = np.asarray(W1, dtype=np.float32)
    b1 = np.asarray(b1, dtype=np.float32)
    W2 = np.asarray(W2, dtype=np.float32)
    b2 = np.asarray(b2, dtype=np.float32)
    Wg = np.asarray(Wg, dtype=np.float32)
    bg = np.asarray(bg, dtype=np.float32)

    noisy = (
        x.astype(np.float64) @ Wg.astype(np.float64)
        + bg.astype(np.float64)
        + 0.1 * noise.astype(np.float64)
    )
    top2 = np.argsort(-noisy, axis=1)[:, :2]

    tok_lists = [np.nonzero((top2 == e).any(axis=1))[0] for e in range(E)]
    max_count = max(len(t) for t in tok_lists)
    slots = ((max_count + 511) // 512) * 512

    in_maps = []
    gathers = []
    for e in range(E):
        toks = tok_lists[e]
        cnt = len(toks)
        padded = np.zeros(slots, dtype=np.int64)
        padded[:cnt] = toks
        xg = x[padded]                              # [slots, D]
        ng = noise[padded]
        sel = np.zeros((slots, E), dtype=np.float32)
        if cnt:
            other = np.where(top2[toks, 0] == e, top2[toks, 1], top2[toks, 0])
            sel[np.arange(cnt), np.full(cnt, e)] = 1.0
            sel[np.arange(cnt), other] = -1.0
        in_maps.append(
            {
                "xTg": np.ascontiguousarray(xg.T),
                "noiseg": np.ascontiguousarray(ng),
                "ohd": sel,
                "W1c": np.ascontiguousarray(W1[e]),
                "b1c": np.ascontiguousarray(b1[e]),
                "W2c": np.ascontiguousarray(W2[e]),
                "b2c": np.ascontiguousarray(b2[e]),
                "Wgc": Wg,
                "bgc": bg,
            }
        )
        gathers.append(toks)
    return in_maps, gathers, slots


def combine(results, gathers):
    """Unshard: scatter-add each core's pre-weighted rows into the output."""
    out = np.zeros((N, D), dtype=np.float32)
    for e in range(E):
        toks = gathers[e]
        out[toks] += results[e]["yc"][: len(toks)]
    return out


def kernel(x, W1, b1, W2, b2, Wg, bg, noise, **_ignored):
    in_maps, gathers, slots = prepare(x, W1, b1, W2, b2, Wg, bg, noise)
    nc = _get_nc(slots)
    res = run_bass_kernel_spmd(nc, in_maps, core_ids=list(range(N_CORES)))
    return combine(res.results, gathers)


# revision 13
# speedup vs baseline: 1.1812x; 1.1812x over previous
"""MoE (noisy top-2 routing, dense expert stack) on 8 Trainium2 NeuronCores.

Strategy: expert-parallel with host-side routing as the sharding step. The
top-2 routing decision is the sharding function: the host computes the noisy
gating in fp64 (bit-robust reproduction of the reference's fp32 top-2
selection), and ships each core exactly the tokens routed to its expert,
padded to a uniform tile count so all 8 cores run the same SPMD program.

Each core then, entirely on device:
  - recomputes the gating logits for its slots (fp32r matmuls) and turns them
    into the top-2 softmax weight of its own expert via a host-provided
    +1/-1 selector plane: w = sigmoid(sum_e noisy[e] * sel[e]) — the
    softmax-over-2 collapses to a sigmoid of the logit difference,
  - runs the expert FFN (relu(x@W1+b1) @ W2 + b2) as fp32r matmuls at full
    PE rate; layer 1 emits h transposed so the ReLU+bias is one activation
    and hT chains straight into layer 2 as lhsT,
  - scales each output row by w and stores the weighted rows.

The host scatter-adds the (at most 2) pre-weighted rows per token — the
"all-reduce of the weighted combine" of the expert-parallel sharding, done
as part of unsharding. Per-core compute drops to the routed ~2/8 of the
dense reference instead of all 8 experts on all tokens.
"""

import sys

sys.path.insert(0, "/opt/trn_rl_repo")

import numpy as np

import concourse.bass as bass
import concourse.mybir as mybir
import concourse.tile as tile
from concourse import bacc
from concourse.bass_utils import run_bass_kernel_spmd

N_CORES = 8
N, D, H, E = 8192, 1024, 2048, 8
P = 128
KD = D // P                 # 8  k-chunks over D
MH = H // P                 # 16 h-chunks

F32 = mybir.dt.float32
F32R = mybir.dt.float32r
AX = mybir.AxisListType
ALU = mybir.AluOpType
ACT_F = mybir.ActivationFunctionType


def _build(slots, repeat=1):
    """SPMD program for one core = one expert over `slots` routed tokens."""
    # Tile widths: full 512-wide tiles plus one 256/384-wide remainder tile
    # (fp32r runs at full PE rate only for free dim >= 256, so slots are
    # rounded so the remainder is 0, 256, or 384).
    assert slots % P == 0 and slots % 512 in (0, 256, 384)
    widths = [512] * (slots // 512)
    if slots % 512:
        widths.append(slots % 512)

    nc = bacc.Bacc(None, target_bir_lowering=False, debug=False)

    xTg = nc.dram_tensor("xTg", [D, slots], F32, kind="ExternalInput")
    noiseg = nc.dram_tensor("noiseg", [slots, E], F32, kind="ExternalInput")
    ohd = nc.dram_tensor("ohd", [slots, E], F32, kind="ExternalInput")
    W1c = nc.dram_tensor("W1c", [D, H], F32, kind="ExternalInput")
    b1c = nc.dram_tensor("b1c", [H], F32, kind="ExternalInput")
    W2c = nc.dram_tensor("W2c", [H, D], F32, kind="ExternalInput")
    b2c = nc.dram_tensor("b2c", [D], F32, kind="ExternalInput")
    Wgc = nc.dram_tensor("Wgc", [D, E], F32, kind="ExternalInput")
    bgc = nc.dram_tensor("bgc", [E], F32, kind="ExternalInput")
    yc = nc.dram_tensor("yc", [slots, D], F32, kind="ExternalOutput")

    with tile.TileContext(nc) as tc:
        with (
            tc.tile_pool(name="persist", bufs=1) as persist,
            tc.tile_pool(name="xs", bufs=2) as xs,
            tc.tile_pool(name="w2s", bufs=3) as w2s,
            tc.tile_pool(name="gat", bufs=2) as gat,
            tc.tile_pool(name="yws", bufs=3) as yws,
            tc.tile_pool(name="pg", bufs=2, space="PSUM") as pg,
            tc.tile_pool(name="ph", bufs=2, space="PSUM") as ph,
            tc.tile_pool(name="py", bufs=4, space="PSUM") as py,
        ):
            # ---- persistent tiles ----
            W1_sb = persist.tile([P, KD, H], F32R)
            nc.sync.dma_start(
                W1_sb[:], W1c.rearrange("(kd p) h -> p kd h", p=P).bitcast(F32R)
            )
            Wg_sb = persist.tile([P, KD, E], F32R)
            nc.sync.dma_start(
                Wg_sb[:], Wgc.rearrange("(kd p) e -> p kd e", p=P).bitcast(F32R)
            )
            b1_sb = persist.tile([P, MH], F32)
            nc.sync.dma_start(b1_sb[:], b1c.rearrange("(m p) -> p m", p=P))
            b2r = persist.tile([P, D], F32)
            nc.sync.dma_start(b2r[:], b2c[None, :].to_broadcast((P, D)))
            bgr = persist.tile([P, E], F32)
            nc.sync.dma_start(bgr[:], bgc[None, :].to_broadcast((P, E)))

            hT = persist.tile([P, MH, 512], F32R)
            wcol = persist.tile([P, slots // P], F32)

            for _rep in range(repeat):
                base = 0
                for st, TW in enumerate(widths):
                    nch = TW // P
                    ss = slice(base, base + TW)
                    xg = xs.tile([P, KD, 512], F32R, tag="xg")
                    nc.sync.dma_start(
                        xg[:, :, :TW],
                        xTg[:, ss].rearrange("(kd p) t -> p kd t", p=P).bitcast(F32R),
                    )
                    noi = xs.tile([P, 4, E], F32, tag="noi")
                    nc.sync.dma_start(
                        noi[:, :nch, :],
                        noiseg[ss, :].rearrange("(c p) e -> p c e", p=P),
                    )
                    oht = xs.tile([P, 4, E], F32, tag="oht")
                    nc.sync.dma_start(
                        oht[:, :nch, :],
                        ohd[ss, :].rearrange("(c p) e -> p c e", p=P),
                    )

                    # gating: w = sigmoid(sum_e sel[e] * noisy[e])
                    for c4 in range(nch):
                        ch = base // P + c4
                        cs = slice(c4 * P, (c4 + 1) * P)
                        g_ps = pg.tile([P, E], F32, tag="gps")
                        for kd in range(KD):
                            nc.tensor.matmul(
                                g_ps[:],
                                xg[:, kd, cs],
                                Wg_sb[:, kd, :],
                                start=(kd == 0),
                                stop=(kd == KD - 1),
                            )
                        noisy = gat.tile([P, E], F32, tag="noisy")
                        nz = gat.tile([P, E], F32, tag="nz")
                        nc.vector.scalar_tensor_tensor(
                            nz[:], noi[:, c4, :], 0.1, bgr[:], ALU.mult, ALU.add
                        )
                        nc.vector.tensor_tensor(noisy[:], g_ps[:], nz[:], ALU.add)
                        sel = gat.tile([P, E], F32, tag="sel")
                        nc.vector.tensor_tensor(
                            sel[:], noisy[:], oht[:, c4, :], ALU.mult
                        )
                        dd = gat.tile([P, 1], F32, tag="dd")
                        nc.vector.tensor_reduce(dd[:], sel[:], axis=AX.X, op=ALU.add)
                        nc.scalar.activation(wcol[:, ch : ch + 1], dd[:], ACT_F.Sigmoid)

                    # layer 1: hT = relu(W1^T-chunk @ x + b1), h on partitions
                    for m in range(MH):
                        h_ps = ph.tile([P, 512], F32, tag="hps")
                        for kd in range(KD):
                            nc.tensor.matmul(
                                h_ps[:, :TW],
                                W1_sb[:, kd, m * P : (m + 1) * P],
                                xg[:, kd, :TW],
                                start=(kd == 0),
                                stop=(kd == KD - 1),
                            )
                        nc.scalar.activation(
                            hT[:, m, :TW],
                            h_ps[:, :TW],
                            ACT_F.Relu,
                            bias=b1_sb[:, m : m + 1],
                        )

                    # layer 2 + bias + weight + store
                    for nh in range(2):
                        ns = slice(nh * 512, (nh + 1) * 512)
                        y_ps = [
                            py.tile([P, 512], F32, tag="yps", name=f"yps{i}")
                            for i in range(4)
                        ]
                        for khg in range(4):
                            w2t = w2s.tile([P, 4, 512], F32R, tag="w2t")
                            nc.sync.dma_start(
                                w2t[:],
                                W2c[khg * 512 : (khg + 1) * 512, ns]
                                .rearrange("(kh p) n -> p kh n", p=P)
                                .bitcast(F32R),
                            )
                            for c4 in range(nch):
                                cs = slice(c4 * P, (c4 + 1) * P)
                                for kh in range(4):
                                    nc.tensor.matmul(
                                        y_ps[c4][:],
                                        hT[:, khg * 4 + kh, cs],
                                        w2t[:, kh, :],
                                        start=(khg == 0 and kh == 0),
                                        stop=(khg == 3 and kh == 3),
                                    )
                        for c4 in range(nch):
                            ch = base // P + c4
                            yw = yws.tile([P, 512], F32, tag="yw")
                            nc.vector.tensor_tensor(
                                yw[:], y_ps[c4][:], b2r[:, ns], ALU.add
                            )
                            nc.vector.tensor_scalar(
                                yw[:], yw[:], wcol[:, ch : ch + 1], None, ALU.mult
                            )
                            nc.sync.dma_start(
                                yc[base + c4 * P : base + (c4 + 1) * P, ns],
                                yw[:],
                            )
                    base += TW

    nc.compile()
    return nc


_NC_CACHE = {}


def _get_nc(slots, repeat=1):
    key = (slots, repeat)
    if key not in _NC_CACHE:
        _NC_CACHE[key] = _build(slots, repeat)
    return _NC_CACHE[key]


def prepare(x, W1, b1, W2, b2, Wg, bg, noise):
    """Host-side routing/sharding: fp64 noisy top-2, per-expert token lists,
    per-core input maps, and the scatter-add spec for unsharding."""
    x = np.ascontiguousarray(np.asarray(x, dtype=np.float32))
    noise = np.asarray(noise, dtype=np.float32)
    W1 = np.asarray(W1, dtype=np.float32)
    b1 = np.asarray(b1, dtype=np.float32)
    W2 = np.asarray(W2, dtype=np.float32)
    b2 = np.asarray(b2, dtype=np.float32)
    Wg = np.asarray(Wg, dtype=np.float32)
    bg = np.asarray(bg, dtype=np.float32)

    noisy = (
        x.astype(np.float64) @ Wg.astype(np.float64)
        + bg.astype(np.float64)
        + 0.1 * noise.astype(np.float64)
    )
    top2 = np.argsort(-noisy, axis=1)[:, :2]

    tok_lists = [np.nonzero((top2 == e).any(axis=1))[0] for e in range(E)]
    max_count = max(len(t) for t in tok_lists)
    slots = ((max_count + P - 1) // P) * P
    if slots % 512 == P:  # remainder tile must be >= 256 for full fp32r rate
        slots += P

    in_maps = []
    gathers = []
    for e in range(E):
        toks = tok_lists[e]
        cnt = len(toks)
        padded = np.zeros(slots, dtype=np.int64)
        padded[:cnt] = toks
        xg = x[padded]                              # [slots, D]
        ng = noise[padded]
        sel = np.zeros((slots, E), dtype=np.float32)
        if cnt:
            other = np.where(top2[toks, 0] == e, top2[toks, 1], top2[toks, 0])
            sel[np.arange(cnt), np.full(cnt, e)] = 1.0
            sel[np.arange(cnt), other] = -1.0
        in_maps.append(
            {
                "xTg": np.ascontiguousarray(xg.T),
                "noiseg": np.ascontiguousarray(ng),
                "ohd": sel,
                "W1c": np.ascontiguousarray(W1[e]),
                "b1c": np.ascontiguousarray(b1[e]),
                "W2c": np.ascontiguousarray(W2[e]),
                "b2c": np.ascontiguousarray(b2[e]),
                "Wgc": Wg,
                "bgc": bg,
            }
        )
        gathers.append(toks)
    return in_maps, gathers, slots


def combine(results, gathers):
    """Unshard: scatter-add each core's pre-weighted rows into the output."""
    out = np.zeros((N, D), dtype=np.float32)
    for e in range(E):
        toks = gathers[e]
        out[toks] += results[e]["yc"][: len(toks)]
    return out


def kernel(x, W1, b1, W2, b2, Wg, bg, noise, **_ignored):
    in_maps, gathers, slots = prepare(x, W1, b1, W2, b2, Wg, bg, noise)
    nc = _get_nc(slots)
    res = run_bass_kernel_spmd(nc, in_maps, core_ids=list(range(N_CORES)))
    return combine(res.results, gathers)
